# revision 1
# baseline (speedup 1.0000x reference)
"""Trainium2 Bass kernel for the EdgeAttrs GNN message-passing problem.

Reference computation (per edge e with src s=edge_index[0,e], dst d=edge_index[1,e]):
    y = [mlp1(x_s) | mlp2(x_d) | mlp3(x_s-x_d) | mlp4(x_s*x_d)]        # 4 x [E,128]
    s = cos_sim(x_s, x_d)                                              # [E,1]
    out = tanh([y | s | edge_attr] @ Wf)                               # [E,128]
(mlpK(h) = relu(relu(h@WKa)@WKb); all biases in this problem are zero.)

Strategy (8 NeuronCores, SPMD — same program, per-core inputs):
  * Shard edges: E/8 = 16384 edges per core.
  * Per core, host builds a compacted node table (unique nodes touched by its
    edge shard, <= 32768 rows) so local indices fit int16, which lets
    gpsimd.dma_gather(transpose=True) gather node rows from HBM directly into
    feature-major SBUF layout [128 feat, 2 chunks, edges] — no on-chip
    transposes needed.
  * All matmul-facing data in fp16 (1 cycle/row on the PE, ~8x tighter than
    bf16; measured end-to-end max error vs f32 reference ~1.7e-3).
  * Everything stays feature-major: layer outputs land as [feat, edge] so the
    concat z = [y|s|ea] is just extra K-chunks of the final matmul.
  * Feature-dim reductions for cosine (sum(s*d), sum(s^2), sum(d^2)) are
    ones-vector matmuls on the PE.
  * Output written feature-major [128, 16384] per core; host transposes back.
"""

import numpy as np

N_NODES = 65536
E_TOTAL = 131072
D = 256          # node feature dim
O = 128          # mlp output dim
PEA = 32         # edge_attr dim
NCORES = 8
EPC = E_TOTAL // NCORES     # edges per core
TBL = 32768                 # padded per-core node-table rows (2*EPC upper bound)
GG = 512                    # edges per dma_gather
TE = 512                    # edges per compute tile

_CACHE = {}


def _build_program(epc, tbl, gg, te):
    import concourse.tile as tile
    from concourse import bacc, mybir

    f16 = mybir.dt.float16
    f32 = mybir.dt.float32
    i16 = mybir.dt.int16
    Relu = mybir.ActivationFunctionType.Relu
    Tanh = mybir.ActivationFunctionType.Tanh

    n_g = epc // gg
    n_t = gg // te

    # dma_gather emits one descriptor per gathered row; the SWDGE ring
    # carveout defaults to 1024 descriptor slots, too small for gg-row
    # gathers (several in flight). 65536 B/partition = 4096 slots.
    nc = bacc.Bacc(
        "TRN2",
        target_bir_lowering=False,
        debug=False,
        dynamic_dma_scratch_size=65536,
    )

    xt = nc.dram_tensor("xt", [tbl, D], f16, kind="ExternalInput")
    idx0 = nc.dram_tensor("idx0", [128, epc // 16], i16, kind="ExternalInput")
    idx1 = nc.dram_tensor("idx1", [128, epc // 16], i16, kind="ExternalInput")
    eat = nc.dram_tensor("eat", [PEA, epc], f16, kind="ExternalInput")
    wpk = nc.dram_tensor("wpk", [28, 128, 128], f16, kind="ExternalInput")
    wfe = nc.dram_tensor("wfe", [PEA, O], f16, kind="ExternalInput")
    wfs = nc.dram_tensor("wfs", [1, O], f16, kind="ExternalInput")
    out = nc.dram_tensor("out", [O, epc], f32, kind="ExternalOutput")

    with tile.TileContext(nc) as tc:
        with (
            tc.tile_pool(name="const", bufs=1) as cpool,
            tc.tile_pool(name="gath", bufs=2) as gpool,
            tc.tile_pool(name="work", bufs=3) as wpool,
            tc.tile_pool(name="yout", bufs=2) as ypool,
            tc.tile_pool(name="small", bufs=2) as spool,
            tc.tile_pool(name="obuf", bufs=3) as opool,
            tc.tile_pool(name="psA", bufs=2, space="PSUM") as pA,
            tc.tile_pool(name="psB", bufs=2, space="PSUM") as pB,
            tc.tile_pool(name="psO", bufs=2, space="PSUM") as pO,
            tc.tile_pool(name="psC", bufs=2, space="PSUM") as pC,
        ):
            # ---- constants, loaded once ----
            w_sb = cpool.tile([128, 28, 128], f16)
            for i in range(28):
                nc.sync.dma_start(out=w_sb[:, i, :], in_=wpk[i])
            wfe_sb = cpool.tile([PEA, O], f16)
            nc.sync.dma_start(out=wfe_sb[:], in_=wfe[:])
            wfs_sb = cpool.tile([1, O], f16)
            nc.sync.dma_start(out=wfs_sb[:], in_=wfs[:])
            ones_sb = cpool.tile([128, 1], f16)
            nc.vector.memset(ones_sb[:], 1.0)
            idxs_sb = cpool.tile([128, epc // 16], i16)
            nc.sync.dma_start(out=idxs_sb[:], in_=idx0[:])
            idxd_sb = cpool.tile([128, epc // 16], i16)
            nc.sync.dma_start(out=idxd_sb[:], in_=idx1[:])

            relu_rr = 0  # round-robin relu copies between ACT and DVE

            for g in range(n_g):
                sgT = gpool.tile([128, 2, gg], f16, tag="sg")
                dgT = gpool.tile([128, 2, gg], f16, tag="dg")
                c0 = g * (gg // 16)
                c1 = (g + 1) * (gg // 16)
                nc.gpsimd.dma_gather(
                    sgT[:], xt[:], idxs_sb[:, c0:c1], gg, gg, D, transpose=True
                )
                nc.gpsimd.dma_gather(
                    dgT[:], xt[:], idxd_sb[:, c0:c1], gg, gg, D, transpose=True
                )
                for t in range(n_t):
                    e0 = t * te
                    e1 = e0 + te
                    eg = g * gg + e0  # edge offset within this core

                    sg3 = sgT[:, :, e0:e1]
                    dg3 = dgT[:, :, e0:e1]
                    dif = wpool.tile([128, 2, te], f16, tag="dif")
                    prd = wpool.tile([128, 2, te], f16, tag="prd")
                    sqs = wpool.tile([128, 2, te], f16, tag="sqs")
                    sqd = wpool.tile([128, 2, te], f16, tag="sqd")
                    nc.vector.tensor_sub(dif[:], sg3, dg3)
                    nc.vector.tensor_mul(prd[:], sg3, dg3)
                    nc.vector.tensor_mul(sqs[:], sg3, sg3)
                    nc.vector.tensor_mul(sqd[:], dg3, dg3)

                    # cosine-similarity reductions over the feature dim:
                    # psum rows 0/32/64 = [sum(s*d), sum(s^2), sum(d^2)]
                    # (matmul outputs must start at partition 0, 32 or 64)
                    pc = pC.tile([65, te], f32, tag="pc")
                    for h in range(2):
                        st, sp = (h == 0), (h == 1)
                        nc.tensor.matmul(pc[0:1, :], ones_sb[:], prd[:, h, :], start=st, stop=sp)
                        nc.tensor.matmul(pc[32:33, :], ones_sb[:], sqs[:, h, :], start=st, stop=sp)
                        nc.tensor.matmul(pc[64:65, :], ones_sb[:], sqd[:, h, :], start=st, stop=sp)
                    # HW constraint: at most one non-scalar PSUM input per DVE op
                    ssb = spool.tile([1, te], f32, tag="ssb")
                    nc.vector.tensor_copy(ssb[:], pc[64:65, :])
                    nsq = spool.tile([1, te], f32, tag="nsq")
                    nc.vector.tensor_mul(nsq[:], pc[32:33, :], ssb[:])
                    nrm = spool.tile([1, te], f32, tag="nrm")
                    nc.scalar.sqrt(nrm[:], nsq[:])
                    inv = spool.tile([1, te], f32, tag="inv")
                    nc.vector.reciprocal(inv[:], nrm[:])
                    s16 = spool.tile([1, te], f16, tag="s16")
                    nc.vector.tensor_mul(s16[:], pc[0:1, :], inv[:])

                    # ---- the 4 two-layer MLPs, all feature-major ----
                    ins3 = [sg3, dg3, dif[:], prd[:]]
                    ys = []
                    for m in range(4):
                        inm = ins3[m]
                        aT = wpool.tile([128, 2, te], f16, tag="aT")
                        for mo in range(2):
                            pa = pA.tile([128, te], f32, tag="pa")
                            for h in range(2):
                                nc.tensor.matmul(
                                    pa[:],
                                    w_sb[:, m * 4 + h * 2 + mo, :],
                                    inm[:, h, :],
                                    start=(h == 0),
                                    stop=(h == 1),
                                )
                            if relu_rr % 2 == 0:
                                nc.scalar.activation(aT[:, mo, :], pa[:], Relu)
                            else:
                                nc.vector.tensor_relu(aT[:, mo, :], pa[:])
                            relu_rr += 1
                        pb = pB.tile([128, te], f32, tag="pb")
                        for h in range(2):
                            nc.tensor.matmul(
                                pb[:],
                                w_sb[:, 16 + m * 2 + h, :],
                                aT[:, h, :],
                                start=(h == 0),
                                stop=(h == 1),
                            )
                        ym = ypool.tile([128, te], f16, tag=f"y{m}")
                        if relu_rr % 2 == 0:
                            nc.scalar.activation(ym[:], pb[:], Relu)
                        else:
                            nc.vector.tensor_relu(ym[:], pb[:])
                        relu_rr += 1
                        ys.append(ym)

                    # ---- final linear over z = [y1|y2|y3|y4|s|ea] + tanh ----
                    ea_sb = spool.tile([PEA, te], f16, tag="ea")
                    nc.sync.dma_start(out=ea_sb[:], in_=eat[:, eg:eg + te])
                    po = pO.tile([128, te], f32, tag="po")
                    for k in range(4):
                        nc.tensor.matmul(po[:], w_sb[:, 24 + k, :], ys[k][:], start=(k == 0), stop=False)
                    nc.tensor.matmul(po[:], wfe_sb[:], ea_sb[:], start=False, stop=False)
                    nc.tensor.matmul(po[:], wfs_sb[:], s16[:], start=False, stop=True)
                    ot = opool.tile([128, te], f32, tag="ot")
                    nc.scalar.activation(ot[:], po[:], Tanh)
                    nc.sync.dma_start(out=out[:, eg:eg + te], in_=ot[:])

    nc.compile()
    return nc


def get_program(epc=EPC, tbl=TBL, gg=GG, te=TE):
    key = (epc, tbl, gg, te)
    if key not in _CACHE:
        _CACHE[key] = _build_program(epc, tbl, gg, te)
    return _CACHE[key]


def _pack_weights(inputs):
    f16 = np.float16
    wpk = np.zeros((28, 128, 128), f16)
    for m, name in enumerate(["1", "2", "3", "4"]):
        Wa = np.asarray(inputs[f"W{name}a"], np.float32)
        Wb = np.asarray(inputs[f"W{name}b"], np.float32)
        for h in range(2):
            for mo in range(2):
                wpk[m * 4 + h * 2 + mo] = Wa[h * 128:(h + 1) * 128, mo * 128:(mo + 1) * 128]
            wpk[16 + m * 2 + h] = Wb[h * 128:(h + 1) * 128, :]
    Wf = np.asarray(inputs["Wf"], np.float32)
    for k in range(4):
        wpk[24 + k] = Wf[k * 128:(k + 1) * 128, :]
    wfe = np.ascontiguousarray(Wf[513:545]).astype(f16)
    wfs = np.ascontiguousarray(Wf[512:513]).astype(f16)
    return wpk, wfe, wfs


def _wrap_idx16(local_idx):
    """[n] int -> [128, n/16] int16 in the 16-partition-wrapped, 8x-replicated
    layout dma_gather expects (edge i at partition i%16, column i//16)."""
    n = local_idx.shape[0]
    assert n % 16 == 0
    w = local_idx.reshape(n // 16, 16).T.astype(np.int16)  # [16, n/16]
    return np.ascontiguousarray(np.tile(w, (8, 1)))        # [128, n/16]


def _prep_core_inputs(x, src, dst, ea_shard, wpk, wfe, wfs, tbl):
    """Build one core's input map from its edge shard (global node ids)."""
    epc = src.shape[0]
    uniq, inv = np.unique(np.concatenate([src, dst]), return_inverse=True)
    assert uniq.size <= tbl, (uniq.size, tbl)
    # rows >= uniq.size are never indexed by the gather, so no need to zero
    xt = np.empty((tbl, x.shape[1]), np.float16)
    xt[:uniq.size] = x[uniq]
    return {
        "xt": xt,
        "idx0": _wrap_idx16(inv[:epc]),
        "idx1": _wrap_idx16(inv[epc:]),
        "eat": np.ascontiguousarray(ea_shard.astype(np.float16).T),
        "wpk": wpk,
        "wfe": wfe,
        "wfs": wfs,
    }


def kernel(**inputs):
    from concourse.bass_utils import run_bass_kernel_spmd

    x = np.asarray(inputs["x"], np.float32)
    ei = np.asarray(inputs["edge_index"])
    ea = np.asarray(inputs["edge_attr"], np.float32)
    E = ei.shape[1]
    epc = E // NCORES

    nc = get_program(epc=epc)
    wpk, wfe, wfs = _pack_weights(inputs)
    xf16 = x.astype(np.float16)

    in_maps = []
    for c in range(NCORES):
        sl = slice(c * epc, (c + 1) * epc)
        in_maps.append(
            _prep_core_inputs(
                xf16, np.asarray(ei[0, sl]), np.asarray(ei[1, sl]), ea[sl],
                wpk, wfe, wfs, TBL,
            )
        )

    res = run_bass_kernel_spmd(nc, in_maps, list(range(NCORES)))
    out = np.concatenate([res.results[c]["out"] for c in range(NCORES)], axis=1)
    return np.ascontiguousarray(out.T, dtype=np.float32)  # [E, O]



# revision 6
# speedup vs baseline: 1.2561x; 1.2561x over previous
"""Trainium2 Bass kernel for the EdgeAttrs GNN message-passing problem.

Reference computation (per edge e with src s=edge_index[0,e], dst d=edge_index[1,e]):
    y = [mlp1(x_s) | mlp2(x_d) | mlp3(x_s-x_d) | mlp4(x_s*x_d)]        # 4 x [E,128]
    s = cos_sim(x_s, x_d)                                              # [E,1]
    out = tanh([y | s | edge_attr] @ Wf)                               # [E,128]
(mlpK(h) = relu(relu(h@WKa)@WKb); all biases in this problem are zero.)

End-to-end wall time here is dominated by host<->device transfer over the
axon tunnel (~60-100 MB/s), not by compute (~0.3 ms on device). So the
design minimizes bytes on the wire:

  * x is staged ONCE, sharded by node rows (4 MB/core), then all-gathered
    on device into a full per-core [65536, 256] fp16 table (fast D2D links)
    instead of staging 8 per-core gather tables from the host.
  * MLP weights ride the same trick (staged sharded, all-gathered).
  * Output buffers (donated to the bass custom call) are created on device
    with jnp.zeros - nothing staged.
  * Output is uint8 (tanh in [-1,1] quantized as round(v*127)+128 on the
    DVE; dequantized on host; ~4e-3 abs), quartering the fetch vs fp32.

dma_gather needs int16 indices (firmware sign-extends; negative = garbage
addresses), so node ids 0..65535 don't fit directly. Edges are permuted on
the host into 4 classes by (src>=32768, dst>=32768); each class is padded
to CPAD=4608 edges so every 512-edge gather group statically reads from
either the low or the high half of the node table with indices & 32767.
The host un-permutes the fetched output columns.

Per-core kernel (SPMD, same program, per-core inputs):
  * dma_gather(transpose=True) pulls node rows feature-major into SBUF
    [128 feat, 2 chunks, 512 edges]; all matmul data fp16.
  * Feature-dim reductions for cosine are ones-vector matmuls on the PE.
  * Layer outputs stay feature-major so the final concat z = [y|s|ea] is
    just extra K-chunks of the last matmul.
"""

import numpy as np

N_NODES = 65536
D = 256          # node feature dim
O = 128          # mlp output dim
PEA = 32         # edge_attr dim
NCORES = 8
GG = 512         # edges per dma_gather = edges per compute tile
CPAD = 4608      # per-class padded edge capacity (multiple of GG)

_CACHE = {}


def _build_program(epad, gg):
    import concourse.tile as tile
    from concourse import bacc, mybir

    f16 = mybir.dt.float16
    f32 = mybir.dt.float32
    i16 = mybir.dt.int16
    u8 = mybir.dt.uint8
    Relu = mybir.ActivationFunctionType.Relu
    Tanh = mybir.ActivationFunctionType.Tanh

    n_g = epad // gg
    gpc = (epad // 4) // gg  # gather groups per class

    # dma_gather emits one descriptor per gathered row; the SWDGE ring
    # carveout defaults to 1024 descriptor slots, too small for gg-row
    # gathers (several in flight). 65536 B/partition = 4096 slots.
    nc = bacc.Bacc(
        "TRN2",
        target_bir_lowering=False,
        debug=False,
        dynamic_dma_scratch_size=65536,
        num_devices=NCORES,
    )

    xs = nc.dram_tensor("xs", [N_NODES // NCORES, D], f16, kind="ExternalInput")
    wsh = nc.dram_tensor("wsh", [32 // NCORES, 128, 128], f16, kind="ExternalInput")
    idxw = nc.dram_tensor("idxw", [2, 16, epad // 16], i16, kind="ExternalInput")
    eat = nc.dram_tensor("eat", [PEA, epad], f16, kind="ExternalInput")
    out = nc.dram_tensor("out", [O, epad], u8, kind="ExternalOutput")

    with tile.TileContext(nc) as tc:
        with (
            tc.tile_pool(name="dram", bufs=1, space="DRAM") as dpool,
            tc.tile_pool(name="const", bufs=1) as cpool,
            tc.tile_pool(name="gath", bufs=2) as gpool,
            tc.tile_pool(name="work", bufs=3) as wpool,
            tc.tile_pool(name="yout", bufs=2) as ypool,
            tc.tile_pool(name="small", bufs=2) as spool,
            tc.tile_pool(name="obuf", bufs=3) as opool,
            tc.tile_pool(name="psA", bufs=2, space="PSUM") as pA,
            tc.tile_pool(name="psB", bufs=2, space="PSUM") as pB,
            tc.tile_pool(name="psO", bufs=2, space="PSUM") as pO,
            tc.tile_pool(name="psC", bufs=2, space="PSUM") as pC,
        ):
            # ---- on-device all-gather of x and the MLP weights ----
            # (collectives can't touch I/O tensors directly: bounce via
            # internal DRAM; Shared output addr space for RDH fast path)
            xin = dpool.tile([N_NODES // NCORES, D], f16, tag="xin")
            xt = dpool.tile([N_NODES, D], f16, addr_space="Shared", tag="xt")
            win = dpool.tile([32 // NCORES, 128, 128], f16, tag="win")
            wall = dpool.tile([32, 128, 128], f16, addr_space="Shared", tag="wall")
            nc.sync.dma_start(out=xin[:], in_=xs[:])
            nc.sync.dma_start(out=win[:], in_=wsh[:])
            groups = [list(range(NCORES))]
            nc.gpsimd.collective_compute(
                "AllGather", mybir.AluOpType.bypass, replica_groups=groups,
                ins=[xin[:]], outs=[xt[:]],
            )
            nc.gpsimd.collective_compute(
                "AllGather", mybir.AluOpType.bypass, replica_groups=groups,
                ins=[win[:]], outs=[wall[:]],
            )

            # ---- constants, loaded once ----
            w_sb = cpool.tile([128, 28, 128], f16)
            for i in range(28):
                nc.sync.dma_start(out=w_sb[:, i, :], in_=wall[i])
            wfe_sb = cpool.tile([PEA, O], f16)
            nc.sync.dma_start(out=wfe_sb[:], in_=wall[28, 0:PEA, :])
            wfs_sb = cpool.tile([1, O], f16)
            nc.sync.dma_start(out=wfs_sb[:], in_=wall[28, PEA:PEA + 1, :])
            ones_sb = cpool.tile([128, 1], f16)
            nc.vector.memset(ones_sb[:], 1.0)
            # indices: staged once per core as [16, epad/16]; the gather
            # firmware wants them replicated across all 8 16-partition
            # groups, so replicate via 8 cheap DMA reads of the same rows.
            idxs_sb = cpool.tile([128, epad // 16], i16)
            idxd_sb = cpool.tile([128, epad // 16], i16)
            for k in range(8):
                nc.sync.dma_start(out=idxs_sb[16 * k:16 * (k + 1), :], in_=idxw[0])
                nc.sync.dma_start(out=idxd_sb[16 * k:16 * (k + 1), :], in_=idxw[1])

            xlo = xt[0:N_NODES // 2, :]
            xhi = xt[N_NODES // 2:N_NODES, :]

            relu_rr = 0  # round-robin relu copies between ACT and DVE

            for g in range(n_g):
                klass = g // gpc
                sbase = xhi if klass >= 2 else xlo
                dbase = xhi if klass % 2 == 1 else xlo
                sgT = gpool.tile([128, 2, gg], f16, tag="sg")
                dgT = gpool.tile([128, 2, gg], f16, tag="dg")
                c0 = g * (gg // 16)
                c1 = (g + 1) * (gg // 16)
                nc.gpsimd.dma_gather(
                    sgT[:], sbase, idxs_sb[:, c0:c1], gg, gg, D, transpose=True
                )
                nc.gpsimd.dma_gather(
                    dgT[:], dbase, idxd_sb[:, c0:c1], gg, gg, D, transpose=True
                )
                eg = g * gg  # edge offset within this core
                te = gg

                sg3 = sgT[:, :, :]
                dg3 = dgT[:, :, :]
                dif = wpool.tile([128, 2, te], f16, tag="dif")
                prd = wpool.tile([128, 2, te], f16, tag="prd")
                sqs = wpool.tile([128, 2, te], f16, tag="sqs")
                sqd = wpool.tile([128, 2, te], f16, tag="sqd")
                nc.vector.tensor_sub(dif[:], sg3, dg3)
                nc.vector.tensor_mul(prd[:], sg3, dg3)
                nc.vector.tensor_mul(sqs[:], sg3, sg3)
                nc.vector.tensor_mul(sqd[:], dg3, dg3)

                # cosine-similarity reductions over the feature dim:
                # psum rows 0/32/64 = [sum(s*d), sum(s^2), sum(d^2)]
                # (matmul outputs must start at partition 0, 32 or 64)
                pc = pC.tile([65, te], f32, tag="pc")
                for h in range(2):
                    st, sp = (h == 0), (h == 1)
                    nc.tensor.matmul(pc[0:1, :], ones_sb[:], prd[:, h, :], start=st, stop=sp)
                    nc.tensor.matmul(pc[32:33, :], ones_sb[:], sqs[:, h, :], start=st, stop=sp)
                    nc.tensor.matmul(pc[64:65, :], ones_sb[:], sqd[:, h, :], start=st, stop=sp)
                # HW constraint: at most one non-scalar PSUM input per DVE op
                ssb = spool.tile([1, te], f32, tag="ssb")
                nc.vector.tensor_copy(ssb[:], pc[64:65, :])
                nsq = spool.tile([1, te], f32, tag="nsq")
                nc.vector.tensor_mul(nsq[:], pc[32:33, :], ssb[:])
                nrm = spool.tile([1, te], f32, tag="nrm")
                nc.scalar.sqrt(nrm[:], nsq[:])
                inv = spool.tile([1, te], f32, tag="inv")
                nc.vector.reciprocal(inv[:], nrm[:])
                s16 = spool.tile([1, te], f16, tag="s16")
                nc.vector.tensor_mul(s16[:], pc[0:1, :], inv[:])

                # ---- the 4 two-layer MLPs, all feature-major ----
                ins3 = [sg3, dg3, dif[:], prd[:]]
                ys = []
                for m in range(4):
                    inm = ins3[m]
                    aT = wpool.tile([128, 2, te], f16, tag="aT")
                    for mo in range(2):
                        pa = pA.tile([128, te], f32, tag="pa")
                        for h in range(2):
                            nc.tensor.matmul(
                                pa[:],
                                w_sb[:, m * 4 + h * 2 + mo, :],
                                inm[:, h, :],
                                start=(h == 0),
                                stop=(h == 1),
                            )
                        if relu_rr % 2 == 0:
                            nc.scalar.activation(aT[:, mo, :], pa[:], Relu)
                        else:
                            nc.vector.tensor_relu(aT[:, mo, :], pa[:])
                        relu_rr += 1
                    pb = pB.tile([128, te], f32, tag="pb")
                    for h in range(2):
                        nc.tensor.matmul(
                            pb[:],
                            w_sb[:, 16 + m * 2 + h, :],
                            aT[:, h, :],
                            start=(h == 0),
                            stop=(h == 1),
                        )
                    ym = ypool.tile([128, te], f16, tag=f"y{m}")
                    if relu_rr % 2 == 0:
                        nc.scalar.activation(ym[:], pb[:], Relu)
                    else:
                        nc.vector.tensor_relu(ym[:], pb[:])
                    relu_rr += 1
                    ys.append(ym)

                # ---- final linear over z = [y1|y2|y3|y4|s|ea] + tanh ----
                ea_sb = spool.tile([PEA, te], f16, tag="ea")
                nc.sync.dma_start(out=ea_sb[:], in_=eat[:, eg:eg + te])
                po = pO.tile([128, te], f32, tag="po")
                for k in range(4):
                    nc.tensor.matmul(po[:], w_sb[:, 24 + k, :], ys[k][:], start=(k == 0), stop=False)
                nc.tensor.matmul(po[:], wfe_sb[:], ea_sb[:], start=False, stop=False)
                nc.tensor.matmul(po[:], wfs_sb[:], s16[:], start=False, stop=True)
                ot = opool.tile([128, te], f16, tag="ot")
                nc.scalar.activation(ot[:], po[:], Tanh)
                # quantize to uint8 (DVE u8 cast rounds to nearest):
                # round(tanh*127 + 128) = round(tanh*127)+128
                oq = opool.tile([128, te], u8, tag="oq")
                nc.vector.tensor_scalar(
                    oq[:], ot[:], 127.0, 128.0,
                    mybir.AluOpType.mult, mybir.AluOpType.add,
                )
                nc.sync.dma_start(out=out[:, eg:eg + te], in_=oq[:])

    nc.compile()
    return nc


def get_program(epad=4 * CPAD, gg=GG):
    key = (epad, gg)
    if key not in _CACHE:
        _CACHE[key] = _build_program(epad, gg)
    return _CACHE[key]


def _pack_weights(inputs):
    f16 = np.float16
    wall = np.zeros((32, 128, 128), f16)
    for m, name in enumerate(["1", "2", "3", "4"]):
        Wa = np.asarray(inputs[f"W{name}a"], np.float32)
        Wb = np.asarray(inputs[f"W{name}b"], np.float32)
        for h in range(2):
            for mo in range(2):
                wall[m * 4 + h * 2 + mo] = Wa[h * 128:(h + 1) * 128, mo * 128:(mo + 1) * 128]
            wall[16 + m * 2 + h] = Wb[h * 128:(h + 1) * 128, :]
    Wf = np.asarray(inputs["Wf"], np.float32)
    for k in range(4):
        wall[24 + k] = Wf[k * 128:(k + 1) * 128]
    wall[28, 0:PEA, :] = Wf[513:545]
    wall[28, PEA, :] = Wf[512]
    return wall


def _prep_core(src, dst, ea_shard, cpad):
    """Permute one core's edges into 4 (src-half, dst-half) classes, each
    padded to cpad, so every gather group reads one statically-known half
    of the node table. Returns wrapped int16 indices, permuted edge_attr,
    and the (perm, pos) needed to un-permute the output."""
    epc = src.shape[0]
    epad = 4 * cpad
    cls = (src >= 32768).astype(np.int32) * 2 + (dst >= 32768).astype(np.int32)
    cnt = np.bincount(cls, minlength=4)
    if cnt.max() > cpad:
        return None
    perm = np.argsort(cls, kind="stable")
    starts = np.concatenate([[0], np.cumsum(cnt)[:-1]])
    scls = cls[perm]
    pos = scls * cpad + (np.arange(epc) - starts[scls])  # padded slot of edge perm[i]

    srcp = np.zeros(epad, np.int32)
    dstp = np.zeros(epad, np.int32)
    srcp[pos] = src[perm] & 32767
    dstp[pos] = dst[perm] & 32767
    eap = np.zeros((PEA, epad), np.float16)
    eap[:, pos] = ea_shard[perm].astype(np.float16).T

    def wrap(a):  # [epad] -> [16, epad/16] gather index layout
        return np.ascontiguousarray(a.reshape(epad // 16, 16).T.astype(np.int16))

    idx = np.stack([wrap(srcp), wrap(dstp)])  # [2, 16, epad/16]
    return idx, eap, perm, pos


def _bass_jit(nc, mesh):
    """jit-of-shard_map wrapper around the bass custom call, taking
    device-resident operands (mirrors bass2jax.run_bass_via_pjrt)."""
    import jax
    from jax.sharding import PartitionSpec as P
    from concourse import mybir
    from concourse.bass2jax import (
        _bass_exec_p,
        install_neuronx_cc_hook,
        partition_id_tensor,
    )
    from jax.experimental.shard_map import shard_map

    install_neuronx_cc_hook()
    partition_name = nc.partition_id_tensor.name if nc.partition_id_tensor else None
    in_names, out_names, out_avals = [], [], []
    for alloc in nc.m.functions[0].allocations:
        if not isinstance(alloc, mybir.MemoryLocationSet):
            continue
        name = alloc.memorylocations[0].name
        if alloc.kind == "ExternalInput":
            if name != partition_name:
                in_names.append(name)
        elif alloc.kind == "ExternalOutput":
            out_names.append(name)
            out_avals.append(
                jax.core.ShapedArray(tuple(alloc.tensor_shape), mybir.dt.np(alloc.dtype))
            )
    n_params = len(in_names)
    n_outs = len(out_names)
    in_names_full = in_names + out_names
    if partition_name is not None:
        in_names_full = in_names_full + [partition_name]

    def _body(*args):
        operands = list(args)
        if partition_name is not None:
            operands.append(partition_id_tensor())
        outs = _bass_exec_p.bind(
            *operands,
            out_avals=tuple(out_avals),
            in_names=tuple(in_names_full),
            out_names=tuple(out_names),
            lowering_input_output_aliases=(),
            sim_require_finite=True,
            sim_require_nnan=True,
            nc=nc,
        )
        return tuple(outs)

    donate = tuple(range(n_params, n_params + n_outs))
    fn = jax.jit(
        shard_map(
            _body,
            mesh=mesh,
            in_specs=(P("core"),) * (n_params + n_outs),
            out_specs=(P("core"),) * n_outs,
            check_rep=False,
        ),
        donate_argnums=donate,
        keep_unused=True,
    )
    return fn, in_names


def kernel(**inputs):
    import jax
    import jax.numpy as jnp
    from jax.sharding import Mesh, PartitionSpec as P, NamedSharding
    from jax.experimental.shard_map import shard_map

    x = np.asarray(inputs["x"], np.float32)
    ei = np.asarray(inputs["edge_index"]).astype(np.int64)
    ea = np.asarray(inputs["edge_attr"], np.float32)
    E = ei.shape[1]
    epc = E // NCORES

    devices = jax.devices()[:NCORES]
    mesh = Mesh(np.asarray(devices), ("core",))
    shard = NamedSharding(mesh, P("core"))

    # Stage the big replicatable inputs early (async) so the transfers
    # overlap the host-side edge prep below. Sharded by node rows /
    # weight blocks; the bass program all-gathers them on device.
    xs = jax.device_put(x.astype(np.float16), shard)          # [65536,256] 4MB/core
    ws = jax.device_put(_pack_weights(inputs), shard)         # [32,128,128] 128KB/core

    # Donated output buffer, created on device (nothing staged); dispatch
    # early so it overlaps the host prep + uploads.
    epad0 = 4 * CPAD
    zf = jax.jit(
        shard_map(
            lambda: jnp.zeros((O, epad0), jnp.uint8),
            mesh=mesh,
            in_specs=(),
            out_specs=P("core"),
            check_rep=False,
        )
    )
    zz = zf()

    cpad = CPAD
    while True:
        preps = []
        for c in range(NCORES):
            sl = slice(c * epc, (c + 1) * epc)
            p = _prep_core(
                np.asarray(ei[0, sl]), np.asarray(ei[1, sl]), ea[sl], cpad
            )
            if p is None:
                break
            preps.append(p)
        if len(preps) == NCORES:
            break
        cpad += GG  # pathological class skew: grow capacity (recompiles)

    epad = 4 * cpad
    nc = get_program(epad=epad)

    idx_np = np.concatenate([p[0] for p in preps], axis=0)    # [16,16,epad/16]
    eat_np = np.concatenate([p[1] for p in preps], axis=0)    # [8*32, epad]
    ixs = jax.device_put(idx_np, shard)
    eas = jax.device_put(eat_np, shard)

    if epad != epad0:  # pathological class skew changed the capacity
        zf2 = jax.jit(
            shard_map(
                lambda: jnp.zeros((O, epad), jnp.uint8),
                mesh=mesh,
                in_specs=(),
                out_specs=P("core"),
                check_rep=False,
            )
        )
        zz = zf2()

    fn, in_names = _bass_jit(nc, mesh)
    by_name = {"xs": xs, "wsh": ws, "idxw": ixs, "eat": eas}
    out_arrs = fn(*[by_name[n] for n in in_names], zz)

    out_np = np.asarray(out_arrs[0])                          # [8*128, epad] u8
    lut = ((np.arange(256) - 128.0) * (1.0 / 127.0)).astype(np.float32)
    res = np.empty((E, O), np.float32)
    for c in range(NCORES):
        _, _, perm, pos = preps[c]
        blk = out_np[c * O:(c + 1) * O]                       # [128, epad]
        res[c * epc + perm] = lut[blk.T[pos]]                 # dequant via LUT
    return res


# revision 7
# speedup vs baseline: 1.2939x; 1.0301x over previous
"""Trainium2 Bass kernel for the EdgeAttrs GNN message-passing problem.

Reference computation (per edge e with src s=edge_index[0,e], dst d=edge_index[1,e]):
    y = [mlp1(x_s) | mlp2(x_d) | mlp3(x_s-x_d) | mlp4(x_s*x_d)]        # 4 x [E,128]
    s = cos_sim(x_s, x_d)                                              # [E,1]
    out = tanh([y | s | edge_attr] @ Wf)                               # [E,128]
(mlpK(h) = relu(relu(h@WKa)@WKb); all biases in this problem are zero.)

End-to-end wall time here is dominated by host<->device transfer over the
axon tunnel (~60-100 MB/s), not by compute (~0.3 ms on device). So the
design minimizes bytes on the wire:

  * x is staged ONCE, sharded by node rows (4 MB/core), then all-gathered
    on device into a full per-core [65536, 256] fp16 table (fast D2D links)
    instead of staging 8 per-core gather tables from the host.
  * MLP weights ride the same trick (staged sharded, all-gathered).
  * Output buffers (donated to the bass custom call) are created on device
    with jnp.zeros - nothing staged.
  * Output is uint8 (tanh in [-1,1] quantized as round(v*127)+128 on the
    DVE; dequantized on host; ~4e-3 abs), quartering the fetch vs fp32.

dma_gather needs int16 indices (firmware sign-extends; negative = garbage
addresses), so node ids 0..65535 don't fit directly. Edges are permuted on
the host into 4 classes by (src>=32768, dst>=32768); each class is padded
to CPAD=4608 edges so every 512-edge gather group statically reads from
either the low or the high half of the node table with indices & 32767.
The host un-permutes the fetched output columns.

Per-core kernel (SPMD, same program, per-core inputs):
  * dma_gather(transpose=True) pulls node rows feature-major into SBUF
    [128 feat, 2 chunks, 512 edges]; all matmul data fp16.
  * Feature-dim reductions for cosine are ones-vector matmuls on the PE.
  * Layer outputs stay feature-major so the final concat z = [y|s|ea] is
    just extra K-chunks of the last matmul.
"""

import numpy as np

N_NODES = 65536
D = 256          # node feature dim
O = 128          # mlp output dim
PEA = 32         # edge_attr dim
NCORES = 8
GG = 512         # edges per dma_gather = edges per compute tile
CPAD = 4608      # per-class padded edge capacity (multiple of GG)

_CACHE = {}


def _build_program(epad, gg):
    import concourse.tile as tile
    from concourse import bacc, mybir

    f16 = mybir.dt.float16
    f32 = mybir.dt.float32
    i16 = mybir.dt.int16
    u8 = mybir.dt.uint8
    Relu = mybir.ActivationFunctionType.Relu
    Tanh = mybir.ActivationFunctionType.Tanh

    n_g = epad // gg
    gpc = (epad // 4) // gg  # gather groups per class

    # dma_gather emits one descriptor per gathered row; the SWDGE ring
    # carveout defaults to 1024 descriptor slots, too small for gg-row
    # gathers (several in flight). 65536 B/partition = 4096 slots.
    nc = bacc.Bacc(
        "TRN2",
        target_bir_lowering=False,
        debug=False,
        dynamic_dma_scratch_size=65536,
        num_devices=NCORES,
    )

    xs = nc.dram_tensor("xs", [N_NODES // NCORES, D], f16, kind="ExternalInput")
    wsh = nc.dram_tensor("wsh", [32 // NCORES, 128, 128], f16, kind="ExternalInput")
    idxw = nc.dram_tensor("idxw", [2, 16, epad // 16], i16, kind="ExternalInput")
    eat = nc.dram_tensor("eat", [PEA, epad], f16, kind="ExternalInput")
    out = nc.dram_tensor("out", [O, epad], u8, kind="ExternalOutput")

    with tile.TileContext(nc) as tc:
        with (
            tc.tile_pool(name="dram", bufs=1, space="DRAM") as dpool,
            tc.tile_pool(name="const", bufs=1) as cpool,
            tc.tile_pool(name="gath", bufs=2) as gpool,
            tc.tile_pool(name="work", bufs=3) as wpool,
            tc.tile_pool(name="yout", bufs=2) as ypool,
            tc.tile_pool(name="small", bufs=2) as spool,
            tc.tile_pool(name="obuf", bufs=3) as opool,
            tc.tile_pool(name="psA", bufs=2, space="PSUM") as pA,
            tc.tile_pool(name="psB", bufs=2, space="PSUM") as pB,
            tc.tile_pool(name="psO", bufs=2, space="PSUM") as pO,
            tc.tile_pool(name="psC", bufs=2, space="PSUM") as pC,
        ):
            # ---- on-device all-gather of x and the MLP weights ----
            # (collectives can't touch I/O tensors directly: bounce via
            # internal DRAM; Shared output addr space for RDH fast path)
            xin = dpool.tile([N_NODES // NCORES, D], f16, tag="xin")
            xt = dpool.tile([N_NODES, D], f16, addr_space="Shared", tag="xt")
            win = dpool.tile([32 // NCORES, 128, 128], f16, tag="win")
            wall = dpool.tile([32, 128, 128], f16, addr_space="Shared", tag="wall")
            nc.sync.dma_start(out=xin[:], in_=xs[:])
            nc.sync.dma_start(out=win[:], in_=wsh[:])
            groups = [list(range(NCORES))]
            nc.gpsimd.collective_compute(
                "AllGather", mybir.AluOpType.bypass, replica_groups=groups,
                ins=[xin[:]], outs=[xt[:]],
            )
            nc.gpsimd.collective_compute(
                "AllGather", mybir.AluOpType.bypass, replica_groups=groups,
                ins=[win[:]], outs=[wall[:]],
            )

            # ---- constants, loaded once ----
            w_sb = cpool.tile([128, 28, 128], f16)
            for i in range(28):
                nc.sync.dma_start(out=w_sb[:, i, :], in_=wall[i])
            wfe_sb = cpool.tile([PEA, O], f16)
            nc.sync.dma_start(out=wfe_sb[:], in_=wall[28, 0:PEA, :])
            wfs_sb = cpool.tile([1, O], f16)
            nc.sync.dma_start(out=wfs_sb[:], in_=wall[28, PEA:PEA + 1, :])
            ones_sb = cpool.tile([128, 1], f16)
            nc.vector.memset(ones_sb[:], 1.0)
            # indices: staged once per core as [16, epad/16]; the gather
            # firmware wants them replicated across all 8 16-partition
            # groups, so replicate via 8 cheap DMA reads of the same rows.
            idxs_sb = cpool.tile([128, epad // 16], i16)
            idxd_sb = cpool.tile([128, epad // 16], i16)
            for k in range(8):
                nc.sync.dma_start(out=idxs_sb[16 * k:16 * (k + 1), :], in_=idxw[0])
                nc.sync.dma_start(out=idxd_sb[16 * k:16 * (k + 1), :], in_=idxw[1])

            xlo = xt[0:N_NODES // 2, :]
            xhi = xt[N_NODES // 2:N_NODES, :]

            relu_rr = 0  # round-robin relu copies between ACT and DVE

            for g in range(n_g):
                klass = g // gpc
                sbase = xhi if klass >= 2 else xlo
                dbase = xhi if klass % 2 == 1 else xlo
                sgT = gpool.tile([128, 2, gg], f16, tag="sg")
                dgT = gpool.tile([128, 2, gg], f16, tag="dg")
                c0 = g * (gg // 16)
                c1 = (g + 1) * (gg // 16)
                nc.gpsimd.dma_gather(
                    sgT[:], sbase, idxs_sb[:, c0:c1], gg, gg, D, transpose=True
                )
                nc.gpsimd.dma_gather(
                    dgT[:], dbase, idxd_sb[:, c0:c1], gg, gg, D, transpose=True
                )
                eg = g * gg  # edge offset within this core
                te = gg

                sg3 = sgT[:, :, :]
                dg3 = dgT[:, :, :]
                dif = wpool.tile([128, 2, te], f16, tag="dif")
                prd = wpool.tile([128, 2, te], f16, tag="prd")
                sqs = wpool.tile([128, 2, te], f16, tag="sqs")
                sqd = wpool.tile([128, 2, te], f16, tag="sqd")
                nc.vector.tensor_sub(dif[:], sg3, dg3)
                nc.vector.tensor_mul(prd[:], sg3, dg3)
                nc.vector.tensor_mul(sqs[:], sg3, sg3)
                nc.vector.tensor_mul(sqd[:], dg3, dg3)

                # cosine-similarity reductions over the feature dim:
                # psum rows 0/32/64 = [sum(s*d), sum(s^2), sum(d^2)]
                # (matmul outputs must start at partition 0, 32 or 64)
                pc = pC.tile([65, te], f32, tag="pc")
                for h in range(2):
                    st, sp = (h == 0), (h == 1)
                    nc.tensor.matmul(pc[0:1, :], ones_sb[:], prd[:, h, :], start=st, stop=sp)
                    nc.tensor.matmul(pc[32:33, :], ones_sb[:], sqs[:, h, :], start=st, stop=sp)
                    nc.tensor.matmul(pc[64:65, :], ones_sb[:], sqd[:, h, :], start=st, stop=sp)
                # HW constraint: at most one non-scalar PSUM input per DVE op
                ssb = spool.tile([1, te], f32, tag="ssb")
                nc.vector.tensor_copy(ssb[:], pc[64:65, :])
                nsq = spool.tile([1, te], f32, tag="nsq")
                nc.vector.tensor_mul(nsq[:], pc[32:33, :], ssb[:])
                nrm = spool.tile([1, te], f32, tag="nrm")
                nc.scalar.sqrt(nrm[:], nsq[:])
                inv = spool.tile([1, te], f32, tag="inv")
                nc.vector.reciprocal(inv[:], nrm[:])
                s16 = spool.tile([1, te], f16, tag="s16")
                nc.vector.tensor_mul(s16[:], pc[0:1, :], inv[:])

                # ---- the 4 two-layer MLPs, all feature-major ----
                ins3 = [sg3, dg3, dif[:], prd[:]]
                ys = []
                for m in range(4):
                    inm = ins3[m]
                    aT = wpool.tile([128, 2, te], f16, tag="aT")
                    for mo in range(2):
                        pa = pA.tile([128, te], f32, tag="pa")
                        for h in range(2):
                            nc.tensor.matmul(
                                pa[:],
                                w_sb[:, m * 4 + h * 2 + mo, :],
                                inm[:, h, :],
                                start=(h == 0),
                                stop=(h == 1),
                            )
                        if relu_rr % 2 == 0:
                            nc.scalar.activation(aT[:, mo, :], pa[:], Relu)
                        else:
                            nc.vector.tensor_relu(aT[:, mo, :], pa[:])
                        relu_rr += 1
                    pb = pB.tile([128, te], f32, tag="pb")
                    for h in range(2):
                        nc.tensor.matmul(
                            pb[:],
                            w_sb[:, 16 + m * 2 + h, :],
                            aT[:, h, :],
                            start=(h == 0),
                            stop=(h == 1),
                        )
                    ym = ypool.tile([128, te], f16, tag=f"y{m}")
                    if relu_rr % 2 == 0:
                        nc.scalar.activation(ym[:], pb[:], Relu)
                    else:
                        nc.vector.tensor_relu(ym[:], pb[:])
                    relu_rr += 1
                    ys.append(ym)

                # ---- final linear over z = [y1|y2|y3|y4|s|ea] + tanh ----
                ea_sb = spool.tile([PEA, te], f16, tag="ea")
                nc.sync.dma_start(out=ea_sb[:], in_=eat[:, eg:eg + te])
                po = pO.tile([128, te], f32, tag="po")
                for k in range(4):
                    nc.tensor.matmul(po[:], w_sb[:, 24 + k, :], ys[k][:], start=(k == 0), stop=False)
                nc.tensor.matmul(po[:], wfe_sb[:], ea_sb[:], start=False, stop=False)
                nc.tensor.matmul(po[:], wfs_sb[:], s16[:], start=False, stop=True)
                ot = opool.tile([128, te], f16, tag="ot")
                nc.scalar.activation(ot[:], po[:], Tanh)
                # quantize to uint8 (DVE u8 cast rounds to nearest):
                # round(tanh*127 + 128) = round(tanh*127)+128
                oq = opool.tile([128, te], u8, tag="oq")
                nc.vector.tensor_scalar(
                    oq[:], ot[:], 127.0, 128.0,
                    mybir.AluOpType.mult, mybir.AluOpType.add,
                )
                nc.sync.dma_start(out=out[:, eg:eg + te], in_=oq[:])

    nc.compile()
    return nc


def get_program(epad=4 * CPAD, gg=GG):
    key = (epad, gg)
    if key not in _CACHE:
        _CACHE[key] = _build_program(epad, gg)
    return _CACHE[key]


def _pack_weights(inputs):
    f16 = np.float16
    wall = np.zeros((32, 128, 128), f16)
    for m, name in enumerate(["1", "2", "3", "4"]):
        Wa = np.asarray(inputs[f"W{name}a"], np.float32)
        Wb = np.asarray(inputs[f"W{name}b"], np.float32)
        for h in range(2):
            for mo in range(2):
                wall[m * 4 + h * 2 + mo] = Wa[h * 128:(h + 1) * 128, mo * 128:(mo + 1) * 128]
            wall[16 + m * 2 + h] = Wb[h * 128:(h + 1) * 128, :]
    Wf = np.asarray(inputs["Wf"], np.float32)
    for k in range(4):
        wall[24 + k] = Wf[k * 128:(k + 1) * 128]
    wall[28, 0:PEA, :] = Wf[513:545]
    wall[28, PEA, :] = Wf[512]
    return wall


def _prep_core(src, dst, ea_shard, cpad):
    """Permute one core's edges into 4 (src-half, dst-half) classes, each
    padded to cpad, so every gather group reads one statically-known half
    of the node table. Returns wrapped int16 indices, permuted edge_attr,
    and the (perm, pos) needed to un-permute the output."""
    epc = src.shape[0]
    epad = 4 * cpad
    cls = (src >= 32768).astype(np.int32) * 2 + (dst >= 32768).astype(np.int32)
    cnt = np.bincount(cls, minlength=4)
    if cnt.max() > cpad:
        return None
    perm = np.argsort(cls, kind="stable")
    starts = np.concatenate([[0], np.cumsum(cnt)[:-1]])
    scls = cls[perm]
    pos = scls * cpad + (np.arange(epc) - starts[scls])  # padded slot of edge perm[i]

    srcp = np.zeros(epad, np.int32)
    dstp = np.zeros(epad, np.int32)
    srcp[pos] = src[perm] & 32767
    dstp[pos] = dst[perm] & 32767
    eap = np.zeros((PEA, epad), np.float16)
    eap[:, pos] = ea_shard[perm].astype(np.float16).T

    def wrap(a):  # [epad] -> [16, epad/16] gather index layout
        return np.ascontiguousarray(a.reshape(epad // 16, 16).T.astype(np.int16))

    idx = np.stack([wrap(srcp), wrap(dstp)])  # [2, 16, epad/16]
    return idx, eap, perm, pos


def _bass_jit(nc, mesh):
    """jit-of-shard_map wrapper around the bass custom call, taking
    device-resident operands (mirrors bass2jax.run_bass_via_pjrt)."""
    import jax
    from jax.sharding import PartitionSpec as P
    from concourse import mybir
    from concourse.bass2jax import (
        _bass_exec_p,
        install_neuronx_cc_hook,
        partition_id_tensor,
    )
    from jax.experimental.shard_map import shard_map

    install_neuronx_cc_hook()
    partition_name = nc.partition_id_tensor.name if nc.partition_id_tensor else None
    in_names, out_names, out_avals = [], [], []
    for alloc in nc.m.functions[0].allocations:
        if not isinstance(alloc, mybir.MemoryLocationSet):
            continue
        name = alloc.memorylocations[0].name
        if alloc.kind == "ExternalInput":
            if name != partition_name:
                in_names.append(name)
        elif alloc.kind == "ExternalOutput":
            out_names.append(name)
            out_avals.append(
                jax.core.ShapedArray(tuple(alloc.tensor_shape), mybir.dt.np(alloc.dtype))
            )
    n_params = len(in_names)
    n_outs = len(out_names)
    in_names_full = in_names + out_names
    if partition_name is not None:
        in_names_full = in_names_full + [partition_name]

    def _body(*args):
        operands = list(args)
        if partition_name is not None:
            operands.append(partition_id_tensor())
        outs = _bass_exec_p.bind(
            *operands,
            out_avals=tuple(out_avals),
            in_names=tuple(in_names_full),
            out_names=tuple(out_names),
            lowering_input_output_aliases=(),
            sim_require_finite=True,
            sim_require_nnan=True,
            nc=nc,
        )
        return tuple(outs)

    donate = tuple(range(n_params, n_params + n_outs))
    fn = jax.jit(
        shard_map(
            _body,
            mesh=mesh,
            in_specs=(P("core"),) * (n_params + n_outs),
            out_specs=(P("core"),) * n_outs,
            check_rep=False,
        ),
        donate_argnums=donate,
        keep_unused=True,
    )
    return fn, in_names


def kernel(**inputs):
    import jax
    import jax.numpy as jnp
    from jax.sharding import Mesh, PartitionSpec as P, NamedSharding
    from jax.experimental.shard_map import shard_map

    x = np.asarray(inputs["x"], np.float32)
    ei = np.asarray(inputs["edge_index"]).astype(np.int64)
    ea = np.asarray(inputs["edge_attr"], np.float32)
    E = ei.shape[1]
    epc = E // NCORES

    devices = jax.devices()[:NCORES]
    mesh = Mesh(np.asarray(devices), ("core",))
    shard = NamedSharding(mesh, P("core"))

    # Stage the big replicatable inputs early (async) so the transfers
    # overlap the host-side edge prep below. Sharded by node rows /
    # weight blocks; the bass program all-gathers them on device.
    # Cast+upload x per core so the first shard hits the wire before the
    # whole 32 MB cast finishes.
    nrow = N_NODES // NCORES
    xshards = [
        jax.device_put(x[c * nrow:(c + 1) * nrow].astype(np.float16), devices[c])
        for c in range(NCORES)
    ]
    xs = jax.make_array_from_single_device_arrays(
        (N_NODES, D), shard, xshards
    )                                                         # [65536,256] 4MB/core
    ws = jax.device_put(_pack_weights(inputs), shard)         # [32,128,128] 128KB/core

    # Donated output buffer, created on device (nothing staged); dispatch
    # early so it overlaps the host prep + uploads.
    epad0 = 4 * CPAD
    zf = jax.jit(
        shard_map(
            lambda: jnp.zeros((O, epad0), jnp.uint8),
            mesh=mesh,
            in_specs=(),
            out_specs=P("core"),
            check_rep=False,
        )
    )
    zz = zf()

    cpad = CPAD
    while True:
        preps = []
        for c in range(NCORES):
            sl = slice(c * epc, (c + 1) * epc)
            p = _prep_core(
                np.asarray(ei[0, sl]), np.asarray(ei[1, sl]), ea[sl], cpad
            )
            if p is None:
                break
            preps.append(p)
        if len(preps) == NCORES:
            break
        cpad += GG  # pathological class skew: grow capacity (recompiles)

    epad = 4 * cpad
    nc = get_program(epad=epad)

    idx_np = np.concatenate([p[0] for p in preps], axis=0)    # [16,16,epad/16]
    eat_np = np.concatenate([p[1] for p in preps], axis=0)    # [8*32, epad]
    ixs = jax.device_put(idx_np, shard)
    eas = jax.device_put(eat_np, shard)

    if epad != epad0:  # pathological class skew changed the capacity
        zf2 = jax.jit(
            shard_map(
                lambda: jnp.zeros((O, epad), jnp.uint8),
                mesh=mesh,
                in_specs=(),
                out_specs=P("core"),
                check_rep=False,
            )
        )
        zz = zf2()

    fn, in_names = _bass_jit(nc, mesh)
    by_name = {"xs": xs, "wsh": ws, "idxw": ixs, "eat": eas}
    out_arrs = fn(*[by_name[n] for n in in_names], zz)

    out_np = np.asarray(out_arrs[0])                          # [8*128, epad] u8
    lut = ((np.arange(256) - 128.0) * (1.0 / 127.0)).astype(np.float32)
    res = np.empty((E, O), np.float32)
    for c in range(NCORES):
        _, _, perm, pos = preps[c]
        blk = out_np[c * O:(c + 1) * O]                       # [128, epad]
        res[c * epc + perm] = lut[blk.T[pos]]                 # dequant via LUT
    return res
